# revision 5
# baseline (speedup 1.0000x reference)
"""Bass/Trainium2 kernel for nn_Channel_attention (bottom-16 channel gather).

reference semantics (per sample b):
    weight = mean(x[b], axis=(H, W))           # [C]
    idx    = argsort(weight)[:16]              # ascending pooled value
    out[b] = x[b, idx]                         # [16, H, W]

Strategy: pure data parallel, B=16 sharded 2 samples per core over 8 cores.
Per core (x shard viewed as [512, 16384] = [(sample, channel), H*W]):
  1. Stream 2 MiB [128ch, 4096] chunks on the SP HWDGE queue (a single
     queue sustains the ~425 GB/s core DMA fabric limit).
  2. Split the pooling reduction across TWO engines: each chunk's two
     2048-col halves go one to DVE reduce_sum and one to the Activation
     engine (activation Copy with accum_out). Each engine runs at ~50%
     duty, so neither ever lags the DMA stream and the select ops slot
     into DVE's idle time.
  3. Per sample: DVE merges partials (negated), PE transposes the group
     sums into a [1, 256] row, two rounds of max8/max_index/match_replace
     give the bottom-16 channel indices in ascending pooled order.
  4. Per round, expand its 8 indices to 16 gather rows (idx*2 + parity,
     two tiny PE matmuls), SWDGE indirect-gather [16, 8192] (32 KiB
     descriptors land one per DMA engine - even spread), and store via
     the Activation engine's HWDGE queue. Round 1's gather+store runs
     while round 2 still selects; sample 0's endgame hides under sample
     1's streaming.
"""

import sys

if "/opt/trn_rl_repo" not in sys.path:
    sys.path.insert(0, "/opt/trn_rl_repo")

import numpy as np

from concourse import bacc, mybir, tile
from concourse.bass import IndirectOffsetOnAxis
from concourse.bass_utils import run_bass_kernel_spmd
from concourse.masks import make_identity

N_CORES = 8
B, C, H, W = 16, 256, 128, 128
K = 16
BPC = B // N_CORES          # samples per core = 2
E = H * W                   # 16384 elems per channel
ROWS = BPC * C              # 512 channel rows per core
NG = C // 128               # channel groups (128 partitions) per sample
GR = 2                      # gather sub-rows per channel (2 x 32 KiB)
GW = E // GR                # gather row width (8192 elems = 32 KiB)
GP = K * GR                 # gather tile partitions (32)

f32 = mybir.dt.float32
i32 = mybir.dt.int32
u32 = mybir.dt.uint32
X = mybir.AxisListType.X
Alu = mybir.AluOpType
ActFn = mybir.ActivationFunctionType

LOAD_W = 4096               # 2 MiB load chunks
RED_W = 2048                # reduce unit; chunks alternate DVE / Act
LOAD_CHUNKS = [4096] * 4
LOAD_CHUNKS_LAST = [4096, 4096, 4096, 2048, 1024, 1024]

_cache = {}


def _units(chunks):
    """(offset, width) reduce units covering the load chunks."""
    units = []
    off = 0
    for w in chunks:
        o = 0
        while o < w:
            uw = min(RED_W, w - o)
            units.append((off + o, uw))
            o += uw
        off += w
    return units


def _build():
    nc = bacc.Bacc("TRN2", target_bir_lowering=False, debug=False,
                   num_devices=N_CORES)
    x_d = nc.dram_tensor("x", [ROWS, E], f32, kind="ExternalInput")
    y_d = nc.dram_tensor("y", [BPC * K * GR, GW], f32,
                         kind="ExternalOutput")

    with tile.TileContext(nc) as tc:
        with (
            tc.tile_pool(name="load", bufs=8) as load_pool,
            tc.tile_pool(name="small", bufs=1) as small,
            tc.tile_pool(name="gather", bufs=1) as gather_pool,
            tc.tile_pool(name="psum", bufs=1, space="PSUM") as psum,
        ):
            # ---- constants (no deps; fill scheduler gaps at startup) ----
            ident = small.tile([128, 128], f32)
            make_identity(nc, ident[:])

            # e_mat[k, j] = (j >> 1 == k): one-hot expansion 8 -> 16 rows
            e_i = small.tile([K, GP], i32)
            nc.gpsimd.iota(out=e_i[:], pattern=[[1, GP]], base=0,
                           channel_multiplier=0)
            nc.vector.tensor_scalar(out=e_i[:], in0=e_i[:], scalar1=1,
                                    scalar2=None, op0=Alu.arith_shift_right)
            e_f = small.tile([K, GP], f32)
            nc.vector.tensor_copy(e_f[:], e_i[:])
            col_i = small.tile([K, 1], i32)
            nc.gpsimd.iota(out=col_i[:], pattern=[[1, 1]], base=0,
                           channel_multiplier=1)
            col_f = small.tile([K, 1], f32)
            nc.vector.tensor_copy(col_f[:], col_i[:])
            e_mat = small.tile([K, GP], f32)
            nc.vector.tensor_scalar(out=e_mat[:], in0=e_f[:],
                                    scalar1=col_f[:], scalar2=None,
                                    op0=Alu.is_equal)

            # a01[p] = p & 1 (gather-row parity)
            pp = small.tile([GP, 1], i32)
            nc.gpsimd.iota(out=pp[:], pattern=[[1, 1]], base=0,
                           channel_multiplier=1)
            nc.vector.tensor_scalar(out=pp[:], in0=pp[:], scalar1=GR - 1,
                                    scalar2=None, op0=Alu.bitwise_and)
            a01 = small.tile([GP, 1], f32)
            nc.vector.tensor_copy(a01[:], pp[:])

            # select scratch (DVE-serial; shared across samples)
            w_neg = small.tile([1, C], f32, tag="wneg")
            w_rep = small.tile([1, C], f32, tag="wrep")
            m1 = small.tile([1, 8], f32, tag="m1")
            m2 = small.tile([1, 8], f32, tag="m2")
            psum_w = psum.tile([1, C], f32, tag="psw")

            xg = x_d[:].rearrange("r (u e) -> (r u) e", u=GR)
            g_tile = gather_pool.tile([GP, GW], f32, tag="g")

            n_unit = 0
            for s in range(BPC):
                idx_u = small.tile([1, K], u32, tag=f"idxu{s}")

                for g in range(NG):
                    last = (s == BPC - 1 and g == NG - 1)
                    chunks = LOAD_CHUNKS_LAST if last else LOAD_CHUNKS
                    units = _units(chunks)
                    base = s * C + g * 128
                    partials = small.tile([128, len(units)], f32,
                                          tag=f"part{s}_{g}")

                    ui = 0
                    off = 0
                    for w in chunks:
                        t = load_pool.tile([128, LOAD_W], f32)
                        nc.sync.dma_start(out=t[:, 0:w],
                                          in_=x_d[base:base + 128,
                                                  off:off + w])
                        coff = off
                        off += w
                        while ui < len(units) and \
                                units[ui][0] + units[ui][1] <= off:
                            uo, uw = units[ui]
                            to = uo - coff
                            if n_unit % 2 == 0:
                                nc.vector.reduce_sum(
                                    out=partials[:, ui:ui + 1],
                                    in_=t[:, to:to + uw], axis=X)
                            else:
                                scr = psum.tile([128, RED_W], f32,
                                                tag="actscr")
                                nc.scalar.activation(
                                    out=scr[:, 0:uw], in_=t[:, to:to + uw],
                                    func=ActFn.Copy,
                                    accum_out=partials[:, ui:ui + 1])
                            n_unit += 1
                            ui += 1

                    vg = small.tile([128, 1], f32, tag=f"v{s}_{g}")
                    nc.vector.reduce_sum(out=vg[:], in_=partials[:],
                                         axis=X, negate=True)
                    nc.tensor.matmul(out=psum_w[:, g * 128:(g + 1) * 128],
                                     lhsT=vg[:], rhs=ident[:],
                                     start=True, stop=True)

                # ---- bottom-16 via two rounds of max8 on -sums ----
                def expand_and_gather(r):
                    # gather row for rank p (p in [0, 16)):
                    # (s*C + idx[8r + (p>>1)])*GR + (p & 1)
                    idx_f = small.tile([1, 8], f32, tag=f"idxf{s}_{r}")
                    nc.vector.tensor_copy(idx_f[:],
                                          idx_u[:, 8 * r:8 * r + 8])
                    psum_t = psum.tile([8, 1], f32, tag="pst")
                    nc.tensor.matmul(out=psum_t[:], lhsT=idx_f[:],
                                     rhs=ident[0:1, 0:1], start=True,
                                     stop=True)
                    idx_t = small.tile([8, 1], f32, tag=f"idxt{s}_{r}")
                    nc.vector.tensor_copy(idx_t[:], psum_t[:])
                    psum_e = psum.tile([K, 1], f32, tag="pse")
                    nc.tensor.matmul(out=psum_e[:], lhsT=e_mat[0:8, 0:K],
                                     rhs=idx_t[:], start=True, stop=True)
                    idx16_f = small.tile([K, 1], f32, tag=f"i16f{s}_{r}")
                    nc.vector.tensor_scalar(out=idx16_f[:], in0=psum_e[:],
                                            scalar1=float(GR),
                                            scalar2=float(s * C * GR),
                                            op0=Alu.mult, op1=Alu.add)
                    idx16_i = small.tile([K, 1], i32, tag=f"i16i{s}_{r}")
                    nc.vector.tensor_tensor(out=idx16_i[:], in0=idx16_f[:],
                                            in1=a01[0:K, :], op=Alu.add)
                    nc.gpsimd.indirect_dma_start(
                        out=g_tile[r * K:(r + 1) * K, :], out_offset=None,
                        in_=xg,
                        in_offset=IndirectOffsetOnAxis(ap=idx16_i[:],
                                                       axis=0))
                    nc.scalar.dma_start(
                        out=y_d[s * GP + r * K:s * GP + (r + 1) * K, :],
                        in_=g_tile[r * K:(r + 1) * K, :])

                nc.vector.tensor_copy(w_neg[:], psum_w[:])
                nc.vector.max(out=m1[:], in_=w_neg[:])
                nc.vector.max_index(out=idx_u[:, 0:8], in_max=m1[:],
                                    in_values=w_neg[:])
                expand_and_gather(0)
                nc.vector.match_replace(out=w_rep[:], in_to_replace=m1[:],
                                        in_values=w_neg[:],
                                        imm_value=-1e38)
                nc.vector.max(out=m2[:], in_=w_rep[:])
                nc.vector.max_index(out=idx_u[:, 8:16], in_max=m2[:],
                                    in_values=w_rep[:])
                expand_and_gather(1)

    nc.compile()
    return nc


def get_nc():
    if "nc" not in _cache:
        _cache["nc"] = _build()
    return _cache["nc"]


def make_in_maps(x: np.ndarray) -> list[dict[str, np.ndarray]]:
    x = np.ascontiguousarray(np.asarray(x, dtype=np.float32))
    assert x.shape == (B, C, H, W)
    return [{"x": x[c * BPC:(c + 1) * BPC].reshape(ROWS, E)}
            for c in range(N_CORES)]


def assemble(results: list[dict[str, np.ndarray]]) -> np.ndarray:
    out = np.empty((B, K, H, W), dtype=np.float32)
    for c in range(N_CORES):
        out[c * BPC:(c + 1) * BPC] = results[c]["y"].reshape(BPC, K, H, W)
    return out


def kernel(x: np.ndarray) -> np.ndarray:
    nc = get_nc()
    res = run_bass_kernel_spmd(nc, make_in_maps(x), list(range(N_CORES)))
    return assemble(res.results)
